# revision 38
# baseline (speedup 1.0000x reference)
"""GPT2 attention (B=2, S=2048, E=1024, H=16) on 8 NeuronCores.

Sharding: tensor-parallel over heads — 2 heads per core. Each core computes
qkv^T for its heads, causal attention in transposed-score layout (k on
partitions, q on free dim), then a partial output projection over its 128
ctx dims. Host sums the 8 partials and adds b_proj.

Restructured for engine overlap vs the phase-serial baseline:
  - Phase C is software-pipelined: scores for chunk i+L are issued before
    the PV matmuls of chunk i, so the PE never waits on the Act-engine exp.
  - Output-projection (phase D) work is queued as small pieces and drained
    one piece per attention chunk, filling the PE bubbles left by the
    Act-bound softmax stream.
  - qkv bias is folded into the PSUM-drain copies (tensor_scalar_add with a
    per-partition bias column) instead of burning PE rows on bias matmuls.
  - Diagonal masking (affine_select) runs on the 128-col diagonal block
    only; stale columns left of it are never read by PV.
  - PSUM->SBUF copies are split between DVE and GpSimd to avoid the DVE
    serial bottleneck on the out projection.
"""
import numpy as np
import ml_dtypes

import concourse.bass as bass
import concourse.bacc as bacc
import concourse.tile as tile
from concourse import mybir
from concourse import masks
from concourse.bass_utils import run_bass_kernel_spmd

BF16 = ml_dtypes.bfloat16
B, S, E, H, D = 2, 2048, 1024, 16, 64
T = B * S                 # 4096 tokens
NCORE = 8
HPC = H // NCORE          # 2 heads per core
NEG = -10000.0
SCALE = D ** -0.5
F32 = mybir.dt.float32
BF = mybir.dt.bfloat16
EXP = mybir.ActivationFunctionType.Exp

_built = {}


def _build():
    if "nc" in _built:
        return _built["nc"]
    nc = bacc.Bacc()
    hsT = nc.declare_dram_parameter("hsT", [E, T], BF, isOutput=False)
    wqkv = nc.declare_dram_parameter("wqkv", [E, 3 * HPC * D], BF, isOutput=False)
    bqkv = nc.declare_dram_parameter("bqkv", [128, 4], F32, isOutput=False)
    wpT = nc.declare_dram_parameter("wpT", [HPC * D, E], BF, isOutput=False)
    padneg = nc.declare_dram_parameter("padneg", [128, 32], F32, isOutput=False)
    out = nc.declare_dram_parameter("out", [T, E], BF, isOutput=True)

    NQ = S // 512             # 4 q-tiles of 512 per batch

    with tile.TileContext(nc) as tc:
        with (
            tc.tile_pool(name="const", bufs=1) as constp,
            tc.tile_pool(name="hst", bufs=8) as hstp,
            tc.tile_pool(name="big", bufs=1) as bigp,
            tc.tile_pool(name="expt", bufs=3) as exptp,
            tc.tile_pool(name="small", bufs=3) as smallp,
            tc.tile_pool(name="outp", bufs=4) as outp,
            tc.tile_pool(name="ps_a", bufs=4, space="PSUM") as ps_a,
            tc.tile_pool(name="ps_b", bufs=3, space="PSUM") as ps_b,
            tc.tile_pool(name="ps_c", bufs=1, space="PSUM") as ps_c,
        ):
            # ---- constants ----
            wqkv_sb = constp.tile([128, 8, 384], BF)
            nc.sync.dma_start(
                out=wqkv_sb, in_=wqkv.rearrange("(kc p) m -> p kc m", p=128)
            )
            bq_sb = constp.tile([128, 4], F32)
            nc.sync.dma_start(out=bq_sb, in_=bqkv[:])
            wpT_sb = constp.tile([128, E], BF)
            nc.sync.dma_start(out=wpT_sb, in_=wpT[:])
            pad_sb = constp.tile([128, 32], F32)
            nc.sync.dma_start(out=pad_sb, in_=padneg[:])
            ident = constp.tile([128, 128], BF)
            masks.make_identity(nc, ident[:])
            zero_sb = constp.tile([128, 128], BF)
            nc.vector.memset(zero_sb, 0.0)

            kT = bigp.tile([128, T], BF)       # rows: h0 dims 0-63, h1 dims 64-127
            # zero-padded per-head q: rows of the OTHER head are zero, so the
            # score matmul can contract over all 128 partitions (the padded
            # rows contribute 0) and match the fast full-width matmul shape.
            qTz = [bigp.tile([128, T], BF, name="qTz0"),
                   bigp.tile([128, T], BF, name="qTz1")]
            nc.vector.memset(qTz[0][64:128, :], 0.0)
            nc.vector.memset(qTz[1][0:64, :], 0.0)
            ctxT = bigp.tile([128, T], BF)
            # v in natural layout: per 128-token chunk tt, 130 cols:
            # [0:64]=h0 dims, [64]=ones, [65:129]=h1 dims, [129]=ones
            vnat = bigp.tile([128, 32, 130], BF)
            ctxn = bigp.tile([128, 32, 128], BF)
            nc.vector.memset(vnat[:, :, 64:65], 1.0)
            nc.vector.memset(vnat[:, :, 129:130], 1.0)

            hsT_r = hsT.rearrange("(kc p) t -> kc p t", p=128)  # [8,128,4096]

            # ---- phase B helpers: qkv^T = Wc @ hsT (bias in drain copies) ----
            def drain_qkv(n, m, pm):
                if m == 0:
                    nc.vector.tensor_scalar_add(
                        qTz[0][0:64, n * 512:(n + 1) * 512], pm[0:64, :],
                        bq_sb[0:64, 0:1])
                    nc.vector.tensor_scalar_add(
                        qTz[1][64:128, n * 512:(n + 1) * 512], pm[64:128, :],
                        bq_sb[64:128, 0:1])
                elif m == 1:
                    nc.vector.tensor_scalar_add(
                        kT[:, n * 512:(n + 1) * 512], pm, bq_sb[:, 1:2])
                else:
                    vtmp = smallp.tile([128, 512], BF, tag="vtmp")
                    nc.vector.tensor_scalar_add(vtmp, pm, bq_sb[:, 2:3])
                    for t in range(4):
                        tt = n * 4 + t
                        pst = ps_b.tile([128, 128], BF, tag="psb", name=f"vt{tt}")
                        nc.tensor.transpose(
                            pst[:], vtmp[:, t * 128:(t + 1) * 128], ident[:])
                        nc.vector.tensor_copy(vnat[:, tt, 0:64], pst[:, 0:64])
                        nc.vector.tensor_copy(vnat[:, tt, 65:129], pst[:, 64:128])

            # batch 0 tiles run up front (attention consumes them immediately)
            for n in range(4):
                pm = [ps_a.tile([128, 512], F32, tag="psa", name=f"qkv{n}_{m}")
                      for m in range(3)]
                for k in range(8):
                    ht = hstp.tile([128, 512], BF, tag="ht")
                    nc.sync.dma_start(out=ht, in_=hsT_r[k, :, n * 512:(n + 1) * 512])
                    for m in range(3):
                        nc.tensor.matmul(
                            pm[m], lhsT=wqkv_sb[:, k, m * 128:(m + 1) * 128], rhs=ht,
                            start=(k == 0), stop=(k == 7),
                        )
                for m in range(3):
                    drain_qkv(n, m, pm[m])

            # ---- phase C: causal attention, software-pipelined ----
            # Scores transposed [k-part, q-free]; PV emits natural-layout ctx
            # [q-part, 65] per 128-q subtile (col 64 = softmax denominator from
            # the ones-column in vnat). Each qs accumulator gets its OWN psum
            # bank (concurrent accumulation groups must not share a bank).
            chunks = []
            for b in range(B):
                for h in range(HPC):
                    for qj in range(NQ):
                        for ki in range(4 * qj + 4):
                            chunks.append((b, h, qj, ki))
            LOOK = 2
            expt_of = {}
            ctxq_of = {}
            # deferred pieces: batch-1 qkv groups first, then phase-D pieces
            pend = [("b", (n, m)) for n in range(4, 8) for m in range(3)]

            def emit_score(c):
                b, h, qj, ki = c
                d = ki - 4 * qj
                hs_, he_ = h * 64, (h + 1) * 64
                if ki == 0:
                    ctxq_of[(b, h, qj)] = [
                        ps_a.tile([128, 512], F32, tag="psa",
                                  name=f"cx{b}{h}{qj}_{qs}")
                        for qs in range(4)]
                scp = ps_b.tile([128, 512], F32, tag="psb", name=f"sc{b}{h}{qj}_{ki}")
                nc.tensor.matmul(
                    scp,
                    lhsT=kT[:, b * S + ki * 128: b * S + (ki + 1) * 128],
                    rhs=qTz[h][:, b * S + qj * 512: b * S + (qj + 1) * 512],
                    start=True, stop=True,
                )
                expt = exptp.tile([128, 512], BF, tag="expt", name=f"ex{b}{h}{qj}_{ki}")
                c0 = 128 * d if d > 0 else 0
                nc.scalar.activation(
                    out=expt[:, c0:], in_=scp[:, c0:], func=EXP,
                    bias=pad_sb[:, b * 16 + ki: b * 16 + ki + 1],
                    scale=SCALE,
                )
                if d >= 0:   # diagonal 128-block: zero where k > q
                    nc.gpsimd.affine_select(
                        out=expt[:, 128 * d:128 * (d + 1)],
                        in_=expt[:, 128 * d:128 * (d + 1)],
                        compare_op=mybir.AluOpType.is_ge, fill=0.0,
                        base=0, channel_multiplier=-1,
                        pattern=[[1, 128]],
                    )
                expt_of[c] = expt

            def emit_pv(c):
                b, h, qj, ki = c
                d = ki - 4 * qj
                expt = expt_of.pop(c)
                ctxq = ctxq_of[(b, h, qj)]
                for qs in range(max(0, d), 4):
                    nc.tensor.matmul(
                        ctxq[qs][:, 0:65],
                        lhsT=expt[:, qs * 128:(qs + 1) * 128],
                        rhs=vnat[:, b * 16 + ki, h * 65:(h + 1) * 65],
                        start=(ki == 0), stop=(ki == 4 * qj + qs),
                    )
                if ki == 4 * qj + 3:
                    emit_norm(b, h, qj)

            def emit_norm(b, h, qj):
                ctxq = ctxq_of.pop((b, h, qj))
                hs_, he_ = h * 64, (h + 1) * 64
                recs = smallp.tile([128, 4], F32, tag="recs", name=f"rec{b}_{h}_{qj}")
                for qs in range(4):
                    nc.vector.reciprocal(recs[:, qs:qs + 1], ctxq[qs][:, 64:65])
                for qs in range(4):
                    tt = b * 16 + qj * 4 + qs
                    nc.vector.tensor_scalar_mul(
                        ctxn[:, tt, hs_:he_], ctxq[qs][:, 0:64],
                        recs[:, qs:qs + 1])
                if h == 1:
                    queue_d(b, qj)

            def emit_piece(kind, tt, drain):
                if kind == "b":
                    # one qkv m-group for a batch-1 tile, hidden in C(b0) slack
                    n, m = tt
                    pm = ps_c.tile([128, 512], F32, tag="psc", name=f"bq{n}_{m}")
                    hts = []
                    for k in range(8):
                        ht = hstp.tile([128, 512], BF, tag="ht")
                        nc.sync.dma_start(
                            out=ht, in_=hsT_r[k, :, n * 512:(n + 1) * 512])
                        hts.append(ht)
                    for k in range(8):
                        nc.tensor.matmul(
                            pm, lhsT=wqkv_sb[:, k, m * 128:(m + 1) * 128],
                            rhs=hts[k], start=(k == 0), stop=(k == 7),
                        )
                    drain_qkv(n, m, pm)
                    return
                if kind == "t":
                    ctp = ps_b.tile([128, 128], BF, tag="psb", name=f"ctp{tt}")
                    nc.tensor.transpose(ctp, ctxn[:, tt, :], ident[:])
                    nc.vector.tensor_copy(ctxT[:, tt * 128:(tt + 1) * 128], ctp)
                    return
                for n2 in range(2):
                    if drain:   # attention done: score/ctx banks are free
                        pp = ps_a.tile([128, 512], F32, tag="psa",
                                       name=f"pp{tt}_{n2}")
                    else:
                        pp = ps_c.tile([128, 512], F32, tag="psc",
                                       name=f"pq{tt}_{n2}")
                    nc.tensor.matmul(
                        pp, lhsT=ctxT[:, tt * 128:(tt + 1) * 128],
                        rhs=wpT_sb[:, n2 * 512:(n2 + 1) * 512],
                        start=True, stop=True,
                    )
                    ot = outp.tile([128, 512], BF, tag="ot")
                    if drain and (tt + n2) % 2:
                        nc.scalar.copy(ot, pp)
                    else:
                        nc.vector.tensor_copy(ot, pp)
                    nc.sync.dma_start(
                        out=out[tt * 128:(tt + 1) * 128,
                                n2 * 512:(n2 + 1) * 512],
                        in_=ot,
                    )

            def queue_d(b, qj):
                # transpose normalized ctx back to [dims, tokens], then the
                # partial out-projection for those tokens; emitted as pieces
                for t in range(4):
                    tt = b * 16 + qj * 4 + t
                    pend.append(("t", tt))
                    pend.append(("d", tt))

            for i in range(len(chunks) + LOOK):
                if i < len(chunks):
                    emit_score(chunks[i])
                j = i - LOOK
                if j >= 0:
                    emit_pv(chunks[j])
                    if pend:
                        emit_piece(*pend.pop(0), False)
                    if len(pend) > 16:
                        emit_piece(*pend.pop(0), False)
            while pend:
                emit_piece(*pend.pop(0), True)
    nc.finalize()
    _built["nc"] = nc
    return nc


def kernel(hidden_states, attention_mask, W_attn, b_attn, W_proj, b_proj,
           _trace=False):
    hs = np.asarray(hidden_states, np.float32).reshape(T, E)
    hsT = np.ascontiguousarray(hs.T).astype(BF16)
    mask = np.asarray(attention_mask)
    padfull = np.where(mask != 0, 0.0, NEG).astype(np.float32)      # [B,S]
    pad = np.ascontiguousarray(
        padfull.reshape(B * 16, 128).T                               # [128, 32]
    )
    W_attn = np.asarray(W_attn, np.float32)
    W_proj = np.asarray(W_proj, np.float32)
    b_attn = np.asarray(b_attn, np.float32)

    in_maps = []
    for c in range(NCORE):
        rows = np.concatenate(
            [np.arange(sec * E + c * 128, sec * E + (c + 1) * 128)
             for sec in range(3)]
        )
        wq = np.ascontiguousarray(W_attn[rows].T).astype(BF16)       # [1024,384]
        bq = np.zeros((128, 4), np.float32)
        bq[:, 0:3] = b_attn[rows].reshape(3, 128).T                  # [128,3]
        wp = np.ascontiguousarray(W_proj[:, c * 128:(c + 1) * 128].T).astype(BF16)
        in_maps.append(
            {"hsT": hsT, "wqkv": wq, "bqkv": bq, "wpT": wp, "padneg": pad}
        )

    nc = _build()
    res = run_bass_kernel_spmd(nc, in_maps, list(range(NCORE)), trace=_trace)
    parts = np.stack([np.asarray(r["out"], np.float32) for r in res.results])
    outv = parts.sum(axis=0) + np.asarray(b_proj, np.float32)[None, :]
    out = outv.reshape(B, S, E).astype(np.float32)
    if _trace:
        return out, res
    return out


# revision 41
# speedup vs baseline: 1.2268x; 1.2268x over previous
"""GPT2 attention (B=2, S=2048, E=1024, H=16) on 8 NeuronCores.

Sharding: tensor-parallel over heads — 2 heads per core. Each core computes
qkv^T for its heads, causal attention in transposed-score layout (k on
partitions, q on free dim), then a partial output projection over its 128
ctx dims. Host sums the 8 partials and adds b_proj.

Restructured for engine overlap vs the phase-serial baseline:
  - Phase C is software-pipelined: scores for chunk i+L are issued before
    the PV matmuls of chunk i, so the PE never waits on the Act-engine exp.
  - Output-projection (phase D) work is queued as small pieces and drained
    one piece per attention chunk, filling the PE bubbles left by the
    Act-bound softmax stream.
  - qkv bias is folded into the PSUM-drain copies (tensor_scalar_add with a
    per-partition bias column) instead of burning PE rows on bias matmuls.
  - Diagonal masking (affine_select) runs on the 128-col diagonal block
    only; stale columns left of it are never read by PV.
  - PSUM->SBUF copies are split between DVE and GpSimd to avoid the DVE
    serial bottleneck on the out projection.
"""
import numpy as np
import ml_dtypes

import concourse.bass as bass
import concourse.bacc as bacc
import concourse.tile as tile
from concourse import mybir
from concourse import masks
from concourse.bass_utils import run_bass_kernel_spmd

BF16 = ml_dtypes.bfloat16
B, S, E, H, D = 2, 2048, 1024, 16, 64
T = B * S                 # 4096 tokens
NCORE = 8
HPC = H // NCORE          # 2 heads per core
NEG = -10000.0
SCALE = D ** -0.5
F32 = mybir.dt.float32
BF = mybir.dt.bfloat16
EXP = mybir.ActivationFunctionType.Exp

_built = {}


def _build():
    if "nc" in _built:
        return _built["nc"]
    nc = bacc.Bacc()
    hsT = nc.declare_dram_parameter("hsT", [E, T], BF, isOutput=False)
    wqkv = nc.declare_dram_parameter("wqkv", [E, 3 * HPC * D], BF, isOutput=False)
    bqkv = nc.declare_dram_parameter("bqkv", [128, 4], F32, isOutput=False)
    wpT = nc.declare_dram_parameter("wpT", [HPC * D, E], BF, isOutput=False)
    padneg = nc.declare_dram_parameter("padneg", [128, 32], F32, isOutput=False)
    out = nc.declare_dram_parameter("out", [T, E], BF, isOutput=True)

    NQ = S // 512             # 4 q-tiles of 512 per batch

    with tile.TileContext(nc) as tc:
        with (
            tc.tile_pool(name="const", bufs=1) as constp,
            tc.tile_pool(name="hst", bufs=8) as hstp,
            tc.tile_pool(name="big", bufs=1) as bigp,
            tc.tile_pool(name="expt", bufs=4) as exptp,
            tc.tile_pool(name="small", bufs=3) as smallp,
            tc.tile_pool(name="outp", bufs=4) as outp,
            tc.tile_pool(name="ps_a", bufs=4, space="PSUM") as ps_a,
            tc.tile_pool(name="ps_b", bufs=3, space="PSUM") as ps_b,
            tc.tile_pool(name="ps_c", bufs=1, space="PSUM") as ps_c,
        ):
            # ---- constants ----
            wqkv_sb = constp.tile([128, 8, 384], BF)
            nc.sync.dma_start(
                out=wqkv_sb, in_=wqkv.rearrange("(kc p) m -> p kc m", p=128)
            )
            bq_sb = constp.tile([128, 4], F32)
            nc.sync.dma_start(out=bq_sb, in_=bqkv[:])
            wpT_sb = constp.tile([128, E], BF)
            nc.sync.dma_start(out=wpT_sb, in_=wpT[:])
            pad_sb = constp.tile([128, 32], F32)
            nc.sync.dma_start(out=pad_sb, in_=padneg[:])
            ident = constp.tile([128, 128], BF)
            masks.make_identity(nc, ident[:])
            zero_sb = constp.tile([128, 128], BF)
            nc.vector.memset(zero_sb, 0.0)

            kT = bigp.tile([128, T], BF)       # rows: h0 dims 0-63, h1 dims 64-127
            # zero-padded per-head q: rows of the OTHER head are zero, so the
            # score matmul can contract over all 128 partitions (the padded
            # rows contribute 0) and match the fast full-width matmul shape.
            qTz = [bigp.tile([128, T], BF, name="qTz0"),
                   bigp.tile([128, T], BF, name="qTz1")]
            nc.vector.memset(qTz[0][64:128, :], 0.0)
            nc.vector.memset(qTz[1][0:64, :], 0.0)
            ctxT = bigp.tile([128, T], BF)
            # v in natural layout: per 128-token chunk tt, 130 cols:
            # [0:64]=h0 dims, [64]=ones, [65:129]=h1 dims, [129]=ones
            vnat = bigp.tile([128, 32, 130], BF)
            ctxn = bigp.tile([128, 32, 128], BF)
            nc.vector.memset(vnat[:, :, 64:65], 1.0)
            nc.vector.memset(vnat[:, :, 129:130], 1.0)

            hsT_r = hsT.rearrange("(kc p) t -> kc p t", p=128)  # [8,128,4096]

            # ---- phase B helpers: qkv^T = Wc @ hsT (bias in drain copies) ----
            def drain_qkv(n, m, pm):
                if m == 0:
                    nc.vector.tensor_scalar_add(
                        qTz[0][0:64, n * 512:(n + 1) * 512], pm[0:64, :],
                        bq_sb[0:64, 0:1])
                    nc.vector.tensor_scalar_add(
                        qTz[1][64:128, n * 512:(n + 1) * 512], pm[64:128, :],
                        bq_sb[64:128, 0:1])
                elif m == 1:
                    nc.vector.tensor_scalar_add(
                        kT[:, n * 512:(n + 1) * 512], pm, bq_sb[:, 1:2])
                else:
                    vtmp = smallp.tile([128, 512], BF, tag="vtmp")
                    nc.vector.tensor_scalar_add(vtmp, pm, bq_sb[:, 2:3])
                    for t in range(4):
                        tt = n * 4 + t
                        pst = ps_b.tile([128, 128], BF, tag="psb", name=f"vt{tt}")
                        nc.tensor.transpose(
                            pst[:], vtmp[:, t * 128:(t + 1) * 128], ident[:])
                        nc.vector.tensor_copy(vnat[:, tt, 0:64], pst[:, 0:64])
                        nc.vector.tensor_copy(vnat[:, tt, 65:129], pst[:, 64:128])

            # batch 0 tiles run up front (attention consumes them immediately)
            for n in range(8):
                pm = [ps_a.tile([128, 512], F32, tag="psa", name=f"qkv{n}_{m}")
                      for m in range(3)]
                for k in range(8):
                    ht = hstp.tile([128, 512], BF, tag="ht")
                    nc.sync.dma_start(out=ht, in_=hsT_r[k, :, n * 512:(n + 1) * 512])
                    for m in range(3):
                        nc.tensor.matmul(
                            pm[m], lhsT=wqkv_sb[:, k, m * 128:(m + 1) * 128], rhs=ht,
                            start=(k == 0), stop=(k == 7),
                        )
                for m in range(3):
                    drain_qkv(n, m, pm[m])

            # ---- phase C: causal attention, software-pipelined ----
            # Scores transposed [k-part, q-free]; PV emits natural-layout ctx
            # [q-part, 65] per 128-q subtile (col 64 = softmax denominator from
            # the ones-column in vnat). Each qs accumulator gets its OWN psum
            # bank (concurrent accumulation groups must not share a bank).
            chunks = []
            for b in range(B):
                for h in range(HPC):
                    for qj in range(NQ):
                        for ki in range(4 * qj + 4):
                            chunks.append((b, h, qj, ki))
            LOOK = 3
            expt_of = {}
            ctxq_of = {}
            pend = []          # deferred phase-D pieces

            def emit_score(c):
                b, h, qj, ki = c
                d = ki - 4 * qj
                hs_, he_ = h * 64, (h + 1) * 64
                if ki == 0:
                    ctxq_of[(b, h, qj)] = [
                        ps_a.tile([128, 512], F32, tag="psa",
                                  name=f"cx{b}{h}{qj}_{qs}")
                        for qs in range(4)]
                scp = ps_b.tile([128, 512], F32, tag="psb", name=f"sc{b}{h}{qj}_{ki}")
                nc.tensor.matmul(
                    scp,
                    lhsT=kT[:, b * S + ki * 128: b * S + (ki + 1) * 128],
                    rhs=qTz[h][:, b * S + qj * 512: b * S + (qj + 1) * 512],
                    start=True, stop=True,
                )
                expt = exptp.tile([128, 512], BF, tag="expt", name=f"ex{b}{h}{qj}_{ki}")
                c0 = 128 * d if d > 0 else 0
                nc.scalar.activation(
                    out=expt[:, c0:], in_=scp[:, c0:], func=EXP,
                    bias=pad_sb[:, b * 16 + ki: b * 16 + ki + 1],
                    scale=SCALE,
                )
                if d >= 0:   # diagonal 128-block: zero where k > q
                    nc.gpsimd.affine_select(
                        out=expt[:, 128 * d:128 * (d + 1)],
                        in_=expt[:, 128 * d:128 * (d + 1)],
                        compare_op=mybir.AluOpType.is_ge, fill=0.0,
                        base=0, channel_multiplier=-1,
                        pattern=[[1, 128]],
                    )
                expt_of[c] = expt

            def emit_pv(c):
                b, h, qj, ki = c
                d = ki - 4 * qj
                expt = expt_of.pop(c)
                ctxq = ctxq_of[(b, h, qj)]
                for qs in range(max(0, d), 4):
                    nc.tensor.matmul(
                        ctxq[qs][:, 0:65],
                        lhsT=expt[:, qs * 128:(qs + 1) * 128],
                        rhs=vnat[:, b * 16 + ki, h * 65:(h + 1) * 65],
                        start=(ki == 0), stop=(ki == 4 * qj + qs),
                    )
                if ki == 4 * qj + 3:
                    emit_norm(b, h, qj)

            def emit_norm(b, h, qj):
                ctxq = ctxq_of.pop((b, h, qj))
                hs_, he_ = h * 64, (h + 1) * 64
                recs = smallp.tile([128, 4], F32, tag="recs", name=f"rec{b}_{h}_{qj}")
                for qs in range(4):
                    nc.vector.reciprocal(recs[:, qs:qs + 1], ctxq[qs][:, 64:65])
                for qs in range(4):
                    tt = b * 16 + qj * 4 + qs
                    nc.vector.tensor_scalar_mul(
                        ctxn[:, tt, hs_:he_], ctxq[qs][:, 0:64],
                        recs[:, qs:qs + 1])
                if h == 1:
                    queue_d(b, qj)

            def emit_piece(kind, tt, drain):
                if kind == "b":
                    # one qkv m-group for a batch-1 tile, hidden in C(b0) slack
                    n, m = tt
                    pm = ps_c.tile([128, 512], F32, tag="psc", name=f"bq{n}_{m}")
                    hts = []
                    for k in range(8):
                        ht = hstp.tile([128, 512], BF, tag="ht")
                        nc.sync.dma_start(
                            out=ht, in_=hsT_r[k, :, n * 512:(n + 1) * 512])
                        hts.append(ht)
                    for k in range(8):
                        nc.tensor.matmul(
                            pm, lhsT=wqkv_sb[:, k, m * 128:(m + 1) * 128],
                            rhs=hts[k], start=(k == 0), stop=(k == 7),
                        )
                    drain_qkv(n, m, pm)
                    return
                if kind == "t":
                    ctp = ps_b.tile([128, 128], BF, tag="psb", name=f"ctp{tt}")
                    nc.tensor.transpose(ctp, ctxn[:, tt, :], ident[:])
                    nc.vector.tensor_copy(ctxT[:, tt * 128:(tt + 1) * 128], ctp)
                    return
                for n2 in range(2):
                    if drain:   # attention done: score/ctx banks are free
                        pp = ps_a.tile([128, 512], F32, tag="psa",
                                       name=f"pp{tt}_{n2}")
                    else:
                        pp = ps_c.tile([128, 512], F32, tag="psc",
                                       name=f"pq{tt}_{n2}")
                    nc.tensor.matmul(
                        pp, lhsT=ctxT[:, tt * 128:(tt + 1) * 128],
                        rhs=wpT_sb[:, n2 * 512:(n2 + 1) * 512],
                        start=True, stop=True,
                    )
                    ot = outp.tile([128, 512], BF, tag="ot")
                    if drain and (tt + n2) % 2:
                        nc.scalar.copy(ot, pp)
                    else:
                        nc.vector.tensor_copy(ot, pp)
                    nc.sync.dma_start(
                        out=out[tt * 128:(tt + 1) * 128,
                                n2 * 512:(n2 + 1) * 512],
                        in_=ot,
                    )

            def queue_d(b, qj):
                # transpose normalized ctx back to [dims, tokens], then the
                # partial out-projection for those tokens; emitted as pieces
                for t in range(4):
                    tt = b * 16 + qj * 4 + t
                    pend.append(("t", tt))
                    pend.append(("d", tt))

            for i in range(len(chunks) + LOOK):
                if i < len(chunks):
                    emit_score(chunks[i])
                j = i - LOOK
                if j >= 0:
                    emit_pv(chunks[j])
                    if pend:
                        emit_piece(*pend.pop(0), False)
                    if len(pend) > 16:
                        emit_piece(*pend.pop(0), False)
            while pend:
                emit_piece(*pend.pop(0), True)
    nc.finalize()
    _built["nc"] = nc
    return nc


def kernel(hidden_states, attention_mask, W_attn, b_attn, W_proj, b_proj,
           _trace=False):
    hs = np.asarray(hidden_states, np.float32).reshape(T, E)
    hsT = np.ascontiguousarray(hs.T).astype(BF16)
    mask = np.asarray(attention_mask)
    padfull = np.where(mask != 0, 0.0, NEG).astype(np.float32)      # [B,S]
    pad = np.ascontiguousarray(
        padfull.reshape(B * 16, 128).T                               # [128, 32]
    )
    W_attn = np.asarray(W_attn, np.float32)
    W_proj = np.asarray(W_proj, np.float32)
    b_attn = np.asarray(b_attn, np.float32)

    in_maps = []
    for c in range(NCORE):
        rows = np.concatenate(
            [np.arange(sec * E + c * 128, sec * E + (c + 1) * 128)
             for sec in range(3)]
        )
        wq = np.ascontiguousarray(W_attn[rows].T).astype(BF16)       # [1024,384]
        bq = np.zeros((128, 4), np.float32)
        bq[:, 0:3] = b_attn[rows].reshape(3, 128).T                  # [128,3]
        wp = np.ascontiguousarray(W_proj[:, c * 128:(c + 1) * 128].T).astype(BF16)
        in_maps.append(
            {"hsT": hsT, "wqkv": wq, "bqkv": bq, "wpT": wp, "padneg": pad}
        )

    nc = _build()
    res = run_bass_kernel_spmd(nc, in_maps, list(range(NCORE)), trace=_trace)
    parts = np.stack([np.asarray(r["out"], np.float32) for r in res.results])
    outv = parts.sum(axis=0) + np.asarray(b_proj, np.float32)[None, :]
    out = outv.reshape(B, S, E).astype(np.float32)
    if _trace:
        return out, res
    return out
